# revision 35
# baseline (speedup 1.0000x reference)
"""Trainium2 Bass kernel for GCNCriticNet (gnn_message_passing).

Structure exploited: 8192 independent complete graphs of 16 nodes (plus GCN
self-loops), so deg=16 everywhere, the sym norm is uniformly 1/16, and the
GCN aggregation collapses to a per-graph mean. Edge lists never reach the
device.

Math per layer l (residual, tanh):
  x_l = tanh( x_{l-1} + (sum_graph x_{l-1}) @ (W_l/16) + b_l )
Head: out_g = (sum_graph x_2) @ w_fc1 / 16 + b_fc1

The kernel is ACT-bound (tanh is scalar-engine-only at ~0.833 ns/elem), so
everything else is kept off ACT and overlapped under it.

Device layout: activations [HID=128 partitions, node free], bf16 (obs is
host-converted to bf16). Per 2048-node chunk (128 graphs):
  - obs lands PRE-TRANSPOSED in SBUF via the DMA xbar (dma_start
    transpose=True) viewing obs as [1024 node-pairs, 128]: obsT col r =
    pair (2r, 2r+1), row f = 64q + o (q = parity). No PE transposes, no
    PSUM staging, half the DMA bytes.
  - x0 = wembQ.T @ obsT (wembA=[We;0] even rows, wembB=[0;We] odd) into a
    PSUM tile with parity-contiguous blocks: chunk col = 512k + 256q +
    128s' + p, graph G = 32k + 16s' + p//8.
  - graph sums of obs via a 3-step pairwise-add cascade over pair columns
    (bf16 2x on DVE/Pool; TensorReduce gets no 2x so cascades win);
    h1n[G,:] = sobs.T @ W01stack (one matmul; W01 = We@W1/16 stacked to
    undo the parity row split); h1 broadcast back to nodes is a second
    PSUM-accumulating matmul with a 0/1 indicator (ind32[G, col]), fused
    onto x0's psum -> u1. tanh1 (ACT, bias folded) -> x1 bf16.
  - sx1 via a 4-step pairwise cascade on DVE (step 1 merges parity);
    h2T = W2s.T @ sx1 (one matmul); u2 = x1 + bcast(h2T) on Pool (two 4D
    adds, one per parity; GPSIMD cannot touch PSUM on real HW, hence the
    h2 psum->sbuf hop on DVE); tanh2 -> x2; sx2 split DVE/Pool; y =
    wfc.T @ sx2 per half.
Software pipeline (Tile scheduler does final ordering): ACT alternates
tanh2(c), tanh1(c+2) so the u2 chain has a full slot of cover; consts ride
one packed DMA; biases DMA'd first (ACT's table load waits on them).
Host: out = y/16 + b_fc1 (graph order is preserved by construction).
"""

import sys
import numpy as np

try:
    import concourse.bass as bass  # noqa: F401
except ImportError:  # harness runs in a bare dir; repo is on the box
    for p in ("/opt/trn_rl_repo", "/root/.axon_site/_ro/trn_rl_repo"):
        if p not in sys.path:
            sys.path.insert(0, p)
    import concourse.bass as bass  # noqa: F401

import ml_dtypes
import concourse.bacc as bacc
import concourse.mybir as mybir
import concourse.tile as tile
from concourse.bass import MemorySpace
from concourse.bass_utils import run_bass_kernel_spmd

F32 = mybir.dt.float32
BF16 = mybir.dt.bfloat16
AF = mybir.ActivationFunctionType
NPBF = ml_dtypes.bfloat16

N_CORES = 8
N_AGENTS = 16
BATCH = 8192
OBS = 64
HID = 128
N = BATCH * N_AGENTS            # 131072 nodes
NPC = N // N_CORES              # 16384 nodes / core
CHUNK = 2048                    # nodes per chunk
NCHUNK = NPC // CHUNK           # 8
SLABS = CHUNK // 256            # 8 slabs of 256 nodes per chunk
GPC = CHUNK // N_AGENTS         # 128 graphs per chunk
OUTPC = NPC // N_AGENTS         # 1024 graphs per core

_CACHE = {}


def _build_nc():
    nc = bacc.Bacc("TRN2", target_bir_lowering=False, debug=False)

    obs_d = nc.dram_tensor("obs", [NPC, OBS], BF16, kind="ExternalInput")
    # packed bf16 consts: w01s | wembA | wembB | w2s | ind32 | wfc(+pad)
    cst_d = nc.dram_tensor("cst", [128, 2576], BF16, kind="ExternalInput")
    bia_d = nc.dram_tensor("bia", [128, 2], F32, kind="ExternalInput")
    out_d = nc.dram_tensor("out", [1, OUTPC], F32, kind="ExternalOutput")

    # DMA-transpose source: row r = node pair (2r, 2r+1); col w = 64q + o
    obs_v = obs_d[:].rearrange(
        "(c r two) o -> c r (two o)", c=NCHUNK, r=1024, two=2
    )

    with tile.TileContext(nc) as tc:
        with (
            tc.tile_pool(name="const", bufs=1) as cp,
            tc.tile_pool(name="io", bufs=5) as iop,
            tc.tile_pool(name="wk", bufs=4) as wp,
            tc.tile_pool(name="act", bufs=4) as ap_,
            tc.tile_pool(name="pu1", bufs=2, space=MemorySpace.PSUM) as pu1p,
            tc.tile_pool(name="psm", bufs=3, space=MemorySpace.PSUM) as psmp,
        ):
            cst = cp.tile([128, 2576], BF16)
            bia = cp.tile([128, 2], F32)
            w01s = cst[:, 0:128]
            wembA = cst[:, 128:256]
            wembB = cst[:, 256:384]
            w2s = cst[:, 384:512]
            wfc = cst[:, 512:513]
            ind32 = cst[:, 528:2576]
            b1f = bia[:, 0:1]
            b2 = bia[:, 1:2]
            outsb = cp.tile([1, OUTPC], F32)

            st_obs = {}
            st_tr = {}
            st_u1 = {}
            st_x1 = {}
            st_u2 = {}

            def stage_dma(c):
                obsT = iop.tile([128, 1024], BF16, tag="obsT")
                nc.sync.dma_start(obsT[:], obs_v[c], transpose=True)
                st_obs[c] = obsT

            def stage_tr(c):
                """graph sums of obs (pair-col cascade) -> h1n_sb."""
                eng = nc.vector if c < 3 else nc.gpsimd
                obsT = st_obs.pop(c)
                smA = psmp.tile([128, 256], F32, tag="sm")
                o1 = wp.tile([128, GPC * 4], BF16, tag="so1")
                v = obsT[:].rearrange("f (g s) -> f g s", s=8)
                eng.tensor_add(
                    o1[:].rearrange("f (g s) -> f g s", s=4),
                    v[:, :, 0:4], v[:, :, 4:8],
                )
                o2 = wp.tile([128, GPC * 2], BF16, tag="so2")
                v = o1[:].rearrange("f (g s) -> f g s", s=4)
                eng.tensor_add(
                    o2[:].rearrange("f (g s) -> f g s", s=2),
                    v[:, :, 0:2], v[:, :, 2:4],
                )
                sobs = wp.tile([128, GPC], BF16, tag="sobs")
                v = o2[:].rearrange("f (g s) -> f g s", s=2)
                eng.tensor_add(
                    sobs[:].rearrange("f (g s) -> f g s", s=1),
                    v[:, :, 0:1], v[:, :, 1:2],
                )
                # h1n[G, hid] = sum_f sobs[f, G] * W01stack[f, hid]
                nc.tensor.matmul(smA[:, 128:256], sobs[:], w01s)
                h1n = wp.tile([GPC, HID], BF16, tag="h1n")
                nc.vector.tensor_copy(h1n[:], smA[:, 128:256])
                st_tr[c] = (obsT, h1n)

            def stage_x0mm(c):
                """x0 + bcast(h1) into psum (PE only)."""
                obsT, h1n = st_tr.pop(c)
                us = []
                for P in range(2):  # two [128,1024] psum piece tiles
                    u1 = pu1p.tile([128, 1024], F32, tag="u1")
                    for m in range(2):  # 512-node half = one psum bank
                        k = 2 * P + m
                        rhs = obsT[:, 256 * k:256 * k + 256]
                        # chunk col = 512k + 256q + 128s' + p
                        for q in range(2):
                            nc.tensor.matmul(
                                u1[:, 512 * m + 256 * q:512 * m + 256 * q + 256],
                                wembA if q == 0 else wembB, rhs,
                                start=(q == 0), stop=False, skip_group_check=True,
                            )
                        nc.tensor.matmul(
                            u1[:, 512 * m:512 * m + 512],
                            h1n[:],
                            cst[:, 528 + 512 * k:1040 + 512 * k],
                            start=False, stop=True, skip_group_check=True,
                        )
                    us.append(u1)
                st_u1[c] = us

            def stage_t1(c):
                """tanh1 -> x1 bf16 (ACT only)."""
                us = st_u1.pop(c)
                x1 = ap_.tile([128, CHUNK], BF16, tag="x1")
                for P in range(2):
                    nc.scalar.activation(
                        x1[:, 1024 * P:1024 * (P + 1)], us[P][:], AF.Tanh,
                        bias=b1f
                    )
                st_x1[c] = x1

            def cascade(x, tag, pool_step1=False):
                """group-16 sums along free dim via pairwise bf16 adds.
                x cols = (k4, q2, s'2, j16, r8); step 1 merges parity."""
                g = GPC
                eng1 = nc.gpsimd if pool_step1 else nc.vector
                t1 = wp.tile([128, g * 8], BF16, tag=tag + "a")
                v = x[:].rearrange("h (k q v) -> h k q v", k=4, q=2)
                eng1.tensor_add(
                    t1[:].rearrange("h (k o v) -> h k o v", k=4, o=1),
                    v[:, :, 0:1, :], v[:, :, 1:2, :],
                )
                t2 = wp.tile([128, g * 4], BF16, tag=tag + "b")
                v = t1[:].rearrange("h (g s) -> h g s", s=8)
                nc.vector.tensor_add(
                    t2[:].rearrange("h (g s) -> h g s", s=4),
                    v[:, :, 0:4], v[:, :, 4:8],
                )
                t3 = wp.tile([128, g * 2], BF16, tag=tag + "c")
                v = t2[:].rearrange("h (g s) -> h g s", s=4)
                nc.vector.tensor_add(
                    t3[:].rearrange("h (g s) -> h g s", s=2),
                    v[:, :, 0:2], v[:, :, 2:4],
                )
                sx = wp.tile([128, g], BF16, tag=tag + "d")
                v = t3[:].rearrange("h (g s) -> h g s", s=2)
                nc.vector.tensor_add(
                    sx[:].rearrange("h (g s) -> h g s", s=1),
                    v[:, :, 0:1], v[:, :, 1:2],
                )
                return sx

            def stage_l2a(c):
                """sx1 cascade, h2, u2 (Pool/DVE/PE — no ACT)."""
                x1 = st_x1.pop(c)
                smB = psmp.tile([128, 256], F32, tag="sm")
                # h2T[hid', G] = sum_hid W2s[hid, hid'] * sx1[hid, G]
                sx1 = cascade(x1, "s1")
                nc.tensor.matmul(smB[:, 0:GPC], w2s, sx1[:])
                h2sb = wp.tile([128, GPC], BF16, tag="h2sb")
                nc.vector.tensor_copy(h2sb[:], smB[:, 0:GPC])
                u2 = ap_.tile([128, CHUNK], BF16, tag="u2")
                u2v = u2[:].rearrange(
                    "h (k q sp j r) -> h k q sp j r", k=4, q=2, sp=2, j=16, r=8
                )
                x1v = x1[:].rearrange(
                    "h (k q sp j r) -> h k q sp j r", k=4, q=2, sp=2, j=16, r=8
                )
                h2b = h2sb[:].rearrange(
                    "h (ks j o) -> h ks j o", ks=8, j=16, o=1
                ).broadcast_to([128, 8, 16, 8]).rearrange(
                    "h (k sp) j r -> h k sp j r", k=4
                )
                for q in range(2):
                    nc.gpsimd.tensor_add(
                        u2v[:, :, q, :, :, :], x1v[:, :, q, :, :, :], h2b
                    )
                st_u2[c] = (u2, smB)

            def half_cascade(eng, x, lo, tag):
                """group-16 sums for cols [lo, lo+1024) of x -> [128, 64]."""
                t1 = wp.tile([128, 512], BF16, tag=tag + "a")
                v = x[:, lo:lo + 1024].rearrange("h (k q v) -> h k q v", k=2, q=2)
                eng.tensor_add(
                    t1[:].rearrange("h (k o v) -> h k o v", k=2, o=1),
                    v[:, :, 0:1, :], v[:, :, 1:2, :],
                )
                t2 = wp.tile([128, 256], BF16, tag=tag + "b")
                v = t1[:].rearrange("h (g s) -> h g s", s=8)
                eng.tensor_add(
                    t2[:].rearrange("h (g s) -> h g s", s=4),
                    v[:, :, 0:4], v[:, :, 4:8],
                )
                t3 = wp.tile([128, 128], BF16, tag=tag + "c")
                v = t2[:].rearrange("h (g s) -> h g s", s=4)
                eng.tensor_add(
                    t3[:].rearrange("h (g s) -> h g s", s=2),
                    v[:, :, 0:2], v[:, :, 2:4],
                )
                sx = wp.tile([128, 64], BF16, tag=tag + "d")
                v = t3[:].rearrange("h (g s) -> h g s", s=2)
                eng.tensor_add(
                    sx[:].rearrange("h (g s) -> h g s", s=1),
                    v[:, :, 0:1], v[:, :, 1:2],
                )
                return sx

            def stage_l2b(c):
                """tanh2 (ACT) then sx2 + head (DVE/Pool split)."""
                u2, smB = st_u2.pop(c)
                x2 = ap_.tile([128, CHUNK], BF16, tag="x2")
                if c == NCHUNK - 1:
                    # drain: split by parity so tanh2a follows u2's q=0 add
                    for q in range(2):
                        nc.scalar.activation(
                            x2[:].rearrange("h (k q v) -> h k q v", k=4, q=2)[
                                :, :, q, :],
                            u2[:].rearrange("h (k q v) -> h k q v", k=4, q=2)[
                                :, :, q, :],
                            AF.Tanh, bias=b2,
                        )
                else:
                    nc.scalar.activation(x2[:], u2[:], AF.Tanh, bias=b2)
                sxa = half_cascade(nc.vector, x2, 0, "s2lo")
                sxb = half_cascade(nc.gpsimd, x2, 1024, "s2hi")
                nc.tensor.matmul(smB[0:1, 128:128 + 64], wfc, sxa[:])
                nc.tensor.matmul(smB[0:1, 192:192 + 64], wfc, sxb[:])
                nc.vector.tensor_copy(
                    outsb[0:1, GPC * c:GPC * (c + 1)], smB[0:1, 128:128 + GPC]
                )

            nc.sync.dma_start(bia[:], bia_d[:])
            stage_dma(0)
            nc.sync.dma_start(cst[:, 0:528], cst_d[:, 0:528])
            nc.sync.dma_start(cst[:, 528:2576], cst_d[:, 528:2576])
            stage_dma(1)
            stage_dma(2)
            stage_dma(3)
            stage_tr(0)
            stage_tr(1)
            stage_x0mm(0)
            stage_t1(0)
            stage_x0mm(1)
            stage_l2a(0)
            stage_t1(1)
            stage_tr(2)
            for c in range(NCHUNK):
                if c + 2 < NCHUNK:
                    stage_x0mm(c + 2)
                if c + 1 < NCHUNK:
                    stage_l2a(c + 1)
                stage_l2b(c)
                if c + 2 < NCHUNK:
                    stage_t1(c + 2)
                if c + 3 < NCHUNK:
                    stage_tr(c + 3)
                if c + 4 < NCHUNK:
                    stage_dma(c + 4)

            nc.sync.dma_start(out_d[:, 0:GPC * 6], outsb[:, 0:GPC * 6])
            nc.sync.dma_start(out_d[:, GPC * 6:], outsb[:, GPC * 6:])

    nc.compile()
    return nc


def _get_nc():
    if "nc" not in _CACHE:
        _CACHE["nc"] = _build_nc()
    return _CACHE["nc"]


def _make_in_maps(cent_obs, w_emb, b_emb, w_gcn, b_gcn, w_fc1):
    w_emb = np.ascontiguousarray(w_emb, np.float32)
    wembz = np.zeros((2, 128, HID), np.float32)
    wembz[0, :OBS] = w_emb
    wembz[1, OBS:] = w_emb
    w01 = w_emb @ (w_gcn[0].astype(np.float32) / np.float32(16.0))
    w01s = np.concatenate([w01, w01], axis=0)       # [128, HID]
    w2s = w_gcn[1].astype(np.float32) / np.float32(16.0)
    b1f = (b_gcn[0] + b_emb + b_emb @ w_gcn[0]).astype(np.float32).reshape(HID, 1)
    b2v = b_gcn[1].astype(np.float32).reshape(HID, 1)
    wfc = w_fc1.astype(np.float32).reshape(HID, 1)
    ind32 = np.zeros((128, 2048), np.float32)
    cols = np.arange(2048)
    # chunk col = 512k + 256q + 128s' + 8j + r -> G = 32k + 16s' + j
    g_of = 32 * (cols // 512) + 16 * ((cols // 128) % 2) + (cols % 128) // 8
    ind32[g_of, cols] = 1.0
    cstv = np.zeros((128, 2576), np.float32)
    cstv[:, 0:128] = w01s
    cstv[:, 128:256] = wembz[0]
    cstv[:, 256:384] = wembz[1]
    cstv[:, 384:512] = w2s
    cstv[:, 512:513] = wfc
    cstv[:, 528:2576] = ind32
    biav = np.concatenate([b1f, b2v], axis=1)       # [128, 2]
    shared = {"cst": cstv.astype(NPBF), "bia": biav.astype(np.float32)}
    in_maps = []
    for ci in range(N_CORES):
        m = dict(shared)
        m["obs"] = np.ascontiguousarray(
            cent_obs[ci * NPC:(ci + 1) * NPC].astype(NPBF)
        )
        in_maps.append(m)
    return in_maps


def kernel(cent_obs, w_emb, b_emb, w_gcn, b_gcn, w_fc1, b_fc1,
           edge_src, edge_dst, _trace=False):
    cent_obs = np.asarray(cent_obs, np.float32)
    nc = _get_nc()
    in_maps = _make_in_maps(
        cent_obs, np.asarray(w_emb, np.float32), np.asarray(b_emb, np.float32),
        np.asarray(w_gcn, np.float32), np.asarray(b_gcn, np.float32),
        np.asarray(w_fc1, np.float32),
    )
    kw = {}
    if _trace:
        kw = dict(trace=True)
    res = run_bass_kernel_spmd(nc, in_maps, list(range(N_CORES)), **kw)
    y = np.concatenate(
        [np.asarray(res.results[i]["out"]).reshape(-1) for i in range(N_CORES)]
    )
    out = (y / np.float32(16.0) + np.float32(np.asarray(b_fc1).reshape(()))).astype(
        np.float32
    )
    if _trace:
        _CACHE["last_result"] = res
    return out.reshape(BATCH, 1)
